# revision 1
# baseline (speedup 1.0000x reference)
"""Complex batch-norm Trainium2 kernel (nn_ComplexBatchNormal).

Full inputs: x_real/x_imag [16, 32, 256, 256] f32, params [32, 256, 256] f32.
Output: complex64 [16, 32, 256, 256].

Sharding: channels C=32 split across 8 cores (4 channels each) -> fully local
batch statistics per core, no collectives.

Per-core algorithm (positions N = 4*256*256 = 262144, batch B = 16):
  pass 1: S_r, S_i, S_rr, S_ii, S_ri per position, accumulated over B via
          TensorE identity-matmuls into PSUM (ScalarE squares, VectorE product).
  coef:   analytic inverse-sqrt of the 2x2 covariance; fold gamma/beta/mu into
          6 per-position coefficients a1,a2,a0,b1,b2,b0 with
          out_r = a1*x_r + a2*x_i + a0, out_i = b1*x_r + b2*x_i + b0.
  pass 2: fp16, batched over half the batch per instruction with step-0
          broadcast APs for the coefficients (DVE 2x mode); bias folded into
          the batched chain; strided fp16->f32 cast-copies (split ScalarE/
          VectorE) emit interleaved (re,im) pairs so the DRAM output is
          directly complex64.
"""

import sys

if "/opt/trn_rl_repo" not in sys.path:
    sys.path.insert(0, "/opt/trn_rl_repo")

from contextlib import ExitStack

import numpy as np

import concourse.bacc as bacc
import concourse.bass as bass
import concourse.tile as tile
from concourse import masks, mybir
from concourse.bass_utils import run_bass_kernel_spmd

P = 128          # SBUF partitions
F = 512          # free-dim positions per tile (= one PSUM bank of f32)
NB = 16          # batch size
HB = NB // 2     # half-batch group for batched pass-2 ops
EPS = 1e-5
N_CORES = 8
C_FULL = 32
C_LOC = C_FULL // N_CORES  # 4 channels per core
HW = 256 * 256
NPOS_FULL = C_LOC * HW     # 262144 positions per core

f32 = mybir.dt.float32
bf16 = mybir.dt.float16  # 16-bit compute dtype for pass 2 (fp16: 10-bit mantissa)


def bcast_free(ap: bass.AP, n: int) -> bass.AP:
    """View [P, F] as [P, n, F] with the middle dim broadcast (step 0)."""
    return bass.AP(tensor=ap.tensor, offset=ap.offset, ap=[ap.ap[0], [0, n], ap.ap[1]])


def _emit(
    nc: bacc.Bacc,
    ctx: ExitStack,
    tc: "tile.TileContext",
    npos: int,
    finals_engine: str = "gpsimd",
):
    NT = npos // (P * F)
    assert NT * P * F == npos

    xr_d = nc.dram_tensor("xr", [NB, npos], f32, kind="ExternalInput")
    xi_d = nc.dram_tensor("xi", [NB, npos], f32, kind="ExternalInput")
    grr_d = nc.dram_tensor("grr", [npos], f32, kind="ExternalInput")
    gri_d = nc.dram_tensor("gri", [npos], f32, kind="ExternalInput")
    gii_d = nc.dram_tensor("gii", [npos], f32, kind="ExternalInput")
    bet_d = nc.dram_tensor("bet", [npos], f32, kind="ExternalInput")
    out_d = nc.dram_tensor("out", [NB, 2 * npos], f32, kind="ExternalOutput")

    G = 4  # batch-samples per load group
    xr_gv = xr_d.ap().rearrange("(g q) (t p f) -> g t p q f", q=G, p=P, f=F)
    xi_gv = xi_d.ap().rearrange("(g q) (t p f) -> g t p q f", q=G, p=P, f=F)
    grr_v = grr_d.ap().rearrange("(t p f) -> t p f", p=P, f=F)
    gri_v = gri_d.ap().rearrange("(t p f) -> t p f", p=P, f=F)
    gii_v = gii_d.ap().rearrange("(t p f) -> t p f", p=P, f=F)
    bet_v = bet_d.ap().rearrange("(t p f) -> t p f", p=P, f=F)
    out_v = out_d.ap().rearrange("b (t p f) -> b t p f", p=P, f=2 * F)

    singles = ctx.enter_context(tc.tile_pool(name="singles", bufs=1))
    xpool = ctx.enter_context(tc.tile_pool(name="x", bufs=2))     # f32 staging groups
    xbpool = ctx.enter_context(tc.tile_pool(name="xb", bufs=2))   # fp16 resident
    sqpool = ctx.enter_context(tc.tile_pool(name="sq", bufs=1))
    gpool = ctx.enter_context(tc.tile_pool(name="g", bufs=1))
    cpool = ctx.enter_context(tc.tile_pool(name="coef", bufs=1))
    cbpool = ctx.enter_context(tc.tile_pool(name="coefb", bufs=1))
    wpool = ctx.enter_context(tc.tile_pool(name="w", bufs=2))
    opool = ctx.enter_context(tc.tile_pool(name="o", bufs=2))
    psum = ctx.enter_context(tc.tile_pool(name="ps", bufs=1, space="PSUM"))

    ident = singles.tile([P, P], f32)
    masks.make_identity(nc, ident[:])
    identb = singles.tile([P, P], bf16)
    nc.scalar.copy(identb[:], ident[:])

    inv16 = 1.0 / NB
    ACT = mybir.ActivationFunctionType

    for t in range(NT):
        # --- params for this position tile ---
        grr = gpool.tile([P, F], f32, tag="grr", name=f"grr{t}")
        gri = gpool.tile([P, F], f32, tag="gri", name=f"gii_{t}_r", bufs=1)
        gii = gpool.tile([P, F], f32, tag="gii", name=f"gii{t}")
        bet = gpool.tile([P, F], f32, tag="bet", name=f"bet{t}")

        # --- pass 1: load x, cast to bf16, accumulate 5 stats over B in PSUM ---
        S_r = psum.tile([P, F], f32, tag="S_r", name=f"S_r{t}")
        S_i = psum.tile([P, F], f32, tag="S_i", name=f"S_i{t}")
        S_rr = psum.tile([P, F], f32, tag="S_rr", name=f"S_rr{t}")
        S_ii = psum.tile([P, F], f32, tag="S_ii", name=f"S_ii{t}")
        S_ri = psum.tile([P, F], f32, tag="S_ri", name=f"S_ri{t}")

        XB = xbpool.tile([P, NB, F], bf16, tag="XB", name=f"XB{t}")
        XIB = xbpool.tile([P, NB, F], bf16, tag="XIB", name=f"XIB{t}")

        for g in range(NB // G):
            xg = xpool.tile([P, G, F], f32, tag="xr", name=f"xr{t}_{g}")
            nc.sync.dma_start(xg[:], xr_gv[g, t])
            yg = xpool.tile([P, G, F], f32, tag="xi", name=f"xi{t}_{g}")
            nc.sync.dma_start(yg[:], xi_gv[g, t])

            XBg = XB[:, g * G : (g + 1) * G, :]
            XIBg = XIB[:, g * G : (g + 1) * G, :]
            nc.scalar.copy(XBg, xg[:])
            nc.vector.tensor_copy(XIBg, yg[:])

            sq_r = sqpool.tile([P, G, F], bf16, tag="sqr", name=f"sqr{t}_{g}")
            sq_i = sqpool.tile([P, G, F], bf16, tag="sqi", name=f"sqi{t}_{g}")
            if t == 0:
                # DVE is idle during the first tile's stats: square there
                nc.vector.tensor_mul(sq_r[:], XBg, XBg)
                nc.vector.tensor_mul(sq_i[:], XIBg, XIBg)
            else:
                nc.scalar.square(sq_r[:], xg[:])
                nc.scalar.square(sq_i[:], yg[:])
            p_g = sqpool.tile([P, G, F], bf16, tag="pg", name=f"pg{t}_{g}")
            nc.vector.tensor_mul(p_g[:], XBg, XIBg)

            for q in range(G):
                b = g * G + q
                st = b == 0
                sp = b == NB - 1
                nc.tensor.matmul(S_r[:], identb[:], XB[:, b, :], start=st, stop=sp)
                nc.tensor.matmul(S_i[:], identb[:], XIB[:, b, :], start=st, stop=sp)
                nc.tensor.matmul(S_rr[:], identb[:], sq_r[:, q, :], start=st, stop=sp)
                nc.tensor.matmul(S_ii[:], identb[:], sq_i[:, q, :], start=st, stop=sp)
                nc.tensor.matmul(S_ri[:], identb[:], p_g[:, q, :], start=st, stop=sp)

        # params arrive after x (not needed until the coefficient phase)
        nc.sync.dma_start(grr[:], grr_v[t])
        nc.sync.dma_start(gri[:], gri_v[t])
        nc.sync.dma_start(gii[:], gii_v[t])
        nc.sync.dma_start(bet[:], bet_v[t])

        # --- coefficient phase (per-position math on [P, F] f32 tiles) ---
        cp = lambda tag: cpool.tile([P, F], f32, tag=tag, name=f"{tag}{t}")
        scr = lambda i: cpool.tile([P, F], f32, tag=f"scr{i}", name=f"scr{i}_{t}")

        mu_r = cp("mu_r")
        nc.scalar.activation(mu_r[:], S_r[:], ACT.Copy, scale=inv16)
        mu_i = cp("mu_i")
        nc.scalar.activation(mu_i[:], S_i[:], ACT.Copy, scale=inv16)
        Vrr = cp("Vrr")
        nc.scalar.activation(Vrr[:], S_rr[:], ACT.Copy, bias=EPS, scale=inv16)
        Vii = cp("Vii")
        nc.scalar.activation(Vii[:], S_ii[:], ACT.Copy, bias=EPS, scale=inv16)
        Vri = cp("Vri")
        nc.scalar.activation(Vri[:], S_ri[:], ACT.Copy, scale=inv16)

        mr2 = scr(0)
        nc.scalar.square(mr2[:], mu_r[:])
        nc.vector.tensor_sub(Vrr[:], Vrr[:], mr2[:])
        mi2 = scr(1)
        nc.scalar.square(mi2[:], mu_i[:])
        nc.vector.tensor_sub(Vii[:], Vii[:], mi2[:])
        mri = scr(2)
        nc.vector.tensor_mul(mri[:], mu_r[:], mu_i[:])
        nc.vector.tensor_sub(Vri[:], Vri[:], mri[:])

        tau = scr(3)
        nc.vector.tensor_add(tau[:], Vrr[:], Vii[:])
        det = scr(4)
        nc.vector.tensor_mul(det[:], Vrr[:], Vii[:])
        vri2 = scr(1)
        nc.scalar.square(vri2[:], Vri[:])
        nc.vector.tensor_sub(det[:], det[:], vri2[:])

        s_s = cp("s_s")
        nc.scalar.sqrt(s_s[:], det[:])
        # tau <- tau + 2*s in one fused op
        nc.vector.scalar_tensor_tensor(
            tau[:], s_s[:], 2.0, tau[:], mybir.AluOpType.mult, mybir.AluOpType.add
        )
        t_t = cp("t_t")
        nc.scalar.sqrt(t_t[:], tau[:])

        st_ = scr(0)
        nc.vector.tensor_mul(st_[:], s_s[:], t_t[:])
        inv = cp("inv")
        nc.vector.reciprocal_approx_fast(inv[:], st_[:])

        # W matrix in place: Wrr <- Vii, Wii <- Vrr, Wri <- Vri
        # (Wri holds +Vri*inv; true Wri = -that)
        nc.vector.tensor_add(Vii[:], Vii[:], s_s[:])
        nc.vector.tensor_mul(Vii[:], Vii[:], inv[:])
        Wrr = Vii
        nc.vector.tensor_add(Vrr[:], Vrr[:], s_s[:])
        nc.vector.tensor_mul(Vrr[:], Vrr[:], inv[:])
        Wii = Vrr
        nc.vector.tensor_mul(Vri[:], Vri[:], inv[:])
        Wri = Vri

        # output coefficients
        a1 = cp("a1")
        nc.vector.tensor_mul(a1[:], grr[:], Wrr[:])
        m2 = cp("m2")
        nc.vector.tensor_mul(m2[:], gri[:], Wri[:])
        nc.vector.tensor_sub(a1[:], a1[:], m2[:])

        a1b = cbpool.tile([P, F], bf16, tag="a1b", name=f"a1b{t}")
        nc.scalar.copy(a1b[:], a1[:])

        a2 = cp("a2")
        nc.vector.tensor_mul(a2[:], gri[:], Wii[:])
        m4 = scr(1)
        nc.vector.tensor_mul(m4[:], grr[:], Wri[:])
        nc.vector.tensor_sub(a2[:], a2[:], m4[:])

        a2b = cbpool.tile([P, F], bf16, tag="a2b", name=f"a2b{t}")
        nc.scalar.copy(a2b[:], a2[:])

        b1 = cp("b1")
        nc.vector.tensor_mul(b1[:], gri[:], Wrr[:])
        m6 = scr(2)
        nc.vector.tensor_mul(m6[:], gii[:], Wri[:])
        nc.vector.tensor_sub(b1[:], b1[:], m6[:])

        b1b = cbpool.tile([P, F], bf16, tag="b1b", name=f"b1b{t}")
        nc.scalar.copy(b1b[:], b1[:])

        b2 = cp("b2")
        nc.vector.tensor_mul(b2[:], gii[:], Wii[:])
        nc.vector.tensor_sub(b2[:], b2[:], m2[:])

        b2b = cbpool.tile([P, F], bf16, tag="b2b", name=f"b2b{t}")
        nc.scalar.copy(b2b[:], b2[:])

        a0 = cpool.tile([P, F], f32, tag="a0", name=f"a0{t}", bufs=2)
        n1 = scr(3)
        nc.vector.tensor_mul(n1[:], a1[:], mu_r[:])
        nc.vector.tensor_sub(a0[:], bet[:], n1[:])
        n2 = scr(4)
        nc.vector.tensor_mul(n2[:], a2[:], mu_i[:])
        nc.vector.tensor_sub(a0[:], a0[:], n2[:])

        a0b = cbpool.tile([P, F], bf16, tag="a0b", name=f"a0b{t}")
        nc.scalar.copy(a0b[:], a0[:])

        b0 = cpool.tile([P, F], f32, tag="b0", name=f"b0{t}", bufs=2)
        n3 = scr(1)
        nc.vector.tensor_mul(n3[:], b1[:], mu_r[:])
        nc.vector.tensor_sub(b0[:], bet[:], n3[:])
        n4 = scr(0)
        nc.vector.tensor_mul(n4[:], b2[:], mu_i[:])
        nc.vector.tensor_sub(b0[:], b0[:], n4[:])


        b0b = cbpool.tile([P, F], bf16, tag="b0b", name=f"b0b{t}")
        nc.scalar.copy(b0b[:], b0[:])

        # --- pass 2: batched bf16, half the batch per instruction ---
        for h in range(2):
            b0_ = h * HB
            XBh = XB[:, b0_ : b0_ + HB, :]
            XIBh = XIB[:, b0_ : b0_ + HB, :]

            U = wpool.tile([P, HB, F], bf16, tag="U", name=f"U{t}_{h}")
            nc.vector.tensor_mul(U[:], XBh, bcast_free(a1b[:], HB))
            V = wpool.tile([P, HB, F], bf16, tag="V", name=f"V{t}_{h}", bufs=2)
            nc.vector.tensor_mul(V[:], XIBh, bcast_free(a2b[:], HB))

            U2 = wpool.tile([P, HB, F], bf16, tag="U", name=f"U2{t}_{h}")
            nc.vector.tensor_mul(U2[:], XBh, bcast_free(b1b[:], HB))
            V2 = wpool.tile([P, HB, F], bf16, tag="V", name=f"V2{t}_{h}", bufs=2)
            nc.vector.tensor_mul(V2[:], XIBh, bcast_free(b2b[:], HB))

            nfin = 0
            for bb in range(HB):
                b = b0_ + bb
                out_c = opool.tile([P, 2 * F], f32, tag="oc", name=f"oc{t}_{b}")
                oc = out_c.rearrange("p (f two) -> p f two", two=2)
                dve_share = 2 if t < NT - 1 else 7
                for comp, (Uc, Vc, cc) in enumerate(((U, V, a0b), (U2, V2, b0b))):
                    # U+V+bias summed on the TensorEngine into PSUM
                    PS = psum.tile(
                        [P, F], f32, tag="PS", name=f"PS{t}_{b}_{comp}", bufs=3
                    )
                    nc.tensor.matmul(
                        PS[:], identb[:], Uc[:, bb, :], start=True, stop=False
                    )
                    nc.tensor.matmul(
                        PS[:], identb[:], Vc[:, bb, :], start=False, stop=False
                    )
                    nc.tensor.matmul(
                        PS[:], identb[:], cc[:], start=False, stop=True
                    )
                    # strided f32 interleave copy from PSUM; split ACT/DVE
                    if nfin % 8 < 8 - dve_share:
                        nc.scalar.copy(oc[:, :, comp], PS[:])
                    else:
                        nc.vector.tensor_copy(oc[:, :, comp], PS[:])
                    nfin += 1
                nc.sync.dma_start(out_v[b, t], out_c[:])


def build_nc(npos: int = NPOS_FULL, finals_engine: str = "gpsimd") -> bacc.Bacc:
    nc = bacc.Bacc("TRN2", target_bir_lowering=False, debug=False)
    with tile.TileContext(nc) as tc:
        with ExitStack() as ctx:
            _emit(nc, ctx, tc, npos, finals_engine=finals_engine)
    nc.compile()
    return nc


_cache: dict = {}


def _get_nc(npos: int, finals_engine: str = "gpsimd") -> bacc.Bacc:
    key = (npos, finals_engine)
    if key not in _cache:
        _cache[key] = build_nc(npos, finals_engine)
    return _cache[key]


def make_in_maps(x_real, x_imag, gamma_rr, gamma_ri, gamma_ii, beta):
    """Shard channels across cores; returns per-core input dicts."""
    in_maps = []
    for k in range(N_CORES):
        sl = slice(k * C_LOC, (k + 1) * C_LOC)
        in_maps.append(
            {
                "xr": np.ascontiguousarray(x_real[:, sl]).reshape(NB, -1),
                "xi": np.ascontiguousarray(x_imag[:, sl]).reshape(NB, -1),
                "grr": np.ascontiguousarray(gamma_rr[sl]).reshape(-1),
                "gri": np.ascontiguousarray(gamma_ri[sl]).reshape(-1),
                "gii": np.ascontiguousarray(gamma_ii[sl]).reshape(-1),
                "bet": np.ascontiguousarray(beta[sl]).reshape(-1),
            }
        )
    return in_maps


def assemble_output(results) -> np.ndarray:
    """Gather per-core interleaved f32 outputs into the full complex64 array."""
    B = NB
    out = np.empty((B, C_FULL, HW), dtype=np.complex64)
    for k in range(N_CORES):
        o = np.asarray(results[k]["out"])  # [B, 2*NPOS] f32
        oc = o.view(np.complex64).reshape(B, C_LOC, HW)
        out[:, k * C_LOC : (k + 1) * C_LOC] = oc
    return out.reshape(B, C_FULL, 256, 256)


def kernel(x_real, x_imag, gamma_rr, gamma_ri, gamma_ii, beta) -> np.ndarray:
    x_real = np.asarray(x_real, dtype=np.float32)
    x_imag = np.asarray(x_imag, dtype=np.float32)
    gamma_rr = np.asarray(gamma_rr, dtype=np.float32)
    gamma_ri = np.asarray(gamma_ri, dtype=np.float32)
    gamma_ii = np.asarray(gamma_ii, dtype=np.float32)
    beta = np.asarray(beta, dtype=np.float32)

    nc = _get_nc(NPOS_FULL)
    in_maps = make_in_maps(x_real, x_imag, gamma_rr, gamma_ri, gamma_ii, beta)
    res = run_bass_kernel_spmd(nc, in_maps, core_ids=list(range(N_CORES)))
    return assemble_output(res.results)



# revision 6
# speedup vs baseline: 1.0064x; 1.0064x over previous
"""Complex batch-norm Trainium2 kernel (nn_ComplexBatchNormal), v2.

Full inputs: x_real/x_imag [16, 32, 256, 256] f32, params [32, 256, 256] f32.
Output: complex64 [16, 32, 256, 256].

Sharding: C=32 channels split over 8 cores (4 each) -> fully local stats.

Device I/O is fp16 (host pre-casts inputs, combines planar fp16 outputs into
complex64; rel-err budget 2e-2 >> fp16 rounding). Per-core HBM traffic is
~38 MB vs ~71 MB for the f32 baseline.

Per tile (positions P*F = 65536, batch 16), engine split:
  ACT    : xr^2, xi^2 (fp16), plus the 1-input coef ops (mu, mu^2, sqrt, cast)
  GpSimd : xr*xi product
  TensorE: all 5 batch-sum stats via fp16 identity-matmul accumulation into
           paired PSUM banks (S_r|S_i, S_rr|S_ii, S_ri)
  DVE    : fused coefficient chain on [P,2,F]/[P,2,2,F] concatenated tiles
           (analytic inverse sqrt of the 2x2 covariance folded with gamma/beta
           into a1,a2,b1,b2,a0,b0), then pass-2 out = a1*xr + a2*xi + a0 (and
           b-side) as broadcast-AP fp16 tensor ops writing fp16 planar outputs.
"""

import sys

if "/opt/trn_rl_repo" not in sys.path:
    sys.path.insert(0, "/opt/trn_rl_repo")

from contextlib import ExitStack

import numpy as np

import concourse.bacc as bacc
import concourse.bass as bass
import concourse.tile as tile
from concourse import masks, mybir
from concourse.bass_utils import run_bass_kernel_spmd

P = 128          # SBUF partitions
F = 512          # free-dim positions per stat tile (= one PSUM bank of f32)
NB = 16          # batch size
NT = 4           # position tiles per core
EPS = 1e-5
N_CORES = 8
C_FULL = 32
C_LOC = C_FULL // N_CORES  # 4 channels per core
HW = 256 * 256
NPOS_FULL = NT * P * F     # 262144 positions per core
G = 4                      # batch group for products / pass-2 chunks

f32 = mybir.dt.float32
f16 = mybir.dt.float16
ALU = mybir.AluOpType
ACTF = mybir.ActivationFunctionType


def bc(ap: bass.AP, n: int, axis: int = 1) -> bass.AP:
    """Insert a step-0 (broadcast) free dim of size n at free-dim `axis`."""
    dims = [list(x) for x in ap.ap]
    return bass.AP(
        tensor=ap.tensor,
        offset=ap.offset,
        ap=dims[:axis] + [[0, n]] + dims[axis:],
    )


def _emit(nc: bacc.Bacc, ctx: ExitStack, tc: "tile.TileContext"):
    xr_d = nc.dram_tensor("xr", [NB, NPOS_FULL], f16, kind="ExternalInput")
    xi_d = nc.dram_tensor("xi", [NB, NPOS_FULL], f16, kind="ExternalInput")
    grr_d = nc.dram_tensor("grr", [NPOS_FULL], f16, kind="ExternalInput")
    gri_d = nc.dram_tensor("gri", [NPOS_FULL], f16, kind="ExternalInput")
    gii_d = nc.dram_tensor("gii", [NPOS_FULL], f16, kind="ExternalInput")
    bet_d = nc.dram_tensor("bet", [NPOS_FULL], f16, kind="ExternalInput")
    outr_d = nc.dram_tensor("outr", [NB, NPOS_FULL], f16, kind="ExternalOutput")
    outi_d = nc.dram_tensor("outi", [NB, NPOS_FULL], f16, kind="ExternalOutput")

    xr_v = xr_d.ap().rearrange("b (t p f) -> t p b f", p=P, f=F)
    xi_v = xi_d.ap().rearrange("b (t p f) -> t p b f", p=P, f=F)
    pv = lambda d: d.ap().rearrange("(t p f) -> t p f", p=P, f=F)
    grr_v, gri_v, gii_v, bet_v = pv(grr_d), pv(gri_d), pv(gii_d), pv(bet_d)
    outr_v = outr_d.ap().rearrange("b (t p f) -> t p b f", p=P, f=F)
    outi_v = outi_d.ap().rearrange("b (t p f) -> t p b f", p=P, f=F)

    singles = ctx.enter_context(tc.tile_pool(name="singles", bufs=1))
    xpool = ctx.enter_context(tc.tile_pool(name="x", bufs=2))
    prodp = ctx.enter_context(tc.tile_pool(name="prod", bufs=2))
    gpool = ctx.enter_context(tc.tile_pool(name="g", bufs=2))
    cpool = ctx.enter_context(tc.tile_pool(name="coef", bufs=1))
    c2pool = ctx.enter_context(tc.tile_pool(name="coef2", bufs=2))
    wpool = ctx.enter_context(tc.tile_pool(name="w", bufs=2))
    opool = ctx.enter_context(tc.tile_pool(name="o", bufs=2))
    psum = ctx.enter_context(tc.tile_pool(name="ps", bufs=1, space="PSUM"))

    identf = singles.tile([P, P], f32)
    masks.make_identity(nc, identf[:])
    identb = singles.tile([P, P], f16)
    nc.scalar.copy(identb[:], identf[:])
    B2EPS = singles.tile([P, 1], f32)
    nc.vector.memset(B2EPS[:], 2.0 * EPS)

    inv16 = 1.0 / NB

    for t in range(NT):
        # ---- load x (fp16), params ----
        XRI = xpool.tile([P, NB, 2, F], f16, tag="XRI", name=f"XRI{t}")
        nc.sync.dma_start(XRI[:, :, 0, :], xr_v[t])
        nc.sync.dma_start(XRI[:, :, 1, :], xi_v[t])

        # G1 = [[grr, gri], [gri, gii]], G2 = [[gri, grr], [gii, gri]]
        G1 = gpool.tile([P, 2, 2, F], f16, tag="G1", name=f"G1_{t}")
        G2 = gpool.tile([P, 2, 2, F], f16, tag="G2", name=f"G2_{t}")
        nc.sync.dma_start(G1[:, 0, 0, :], grr_v[t])
        nc.sync.dma_start(G1[:, 0, 1, :], gri_v[t])
        nc.sync.dma_start(G1[:, 1, 0, :], gri_v[t])
        nc.sync.dma_start(G1[:, 1, 1, :], gii_v[t])
        nc.sync.dma_start(G2[:, 0, 0, :], gri_v[t])
        nc.sync.dma_start(G2[:, 0, 1, :], grr_v[t])
        nc.sync.dma_start(G2[:, 1, 0, :], gii_v[t])
        nc.sync.dma_start(G2[:, 1, 1, :], gri_v[t])
        BET = gpool.tile([P, F], f16, tag="BET", name=f"BET{t}")
        nc.sync.dma_start(BET[:], bet_v[t])

        # ---- pass 1: stats. products on ACT/GpSimd, sums on TensorE ----
        SR2 = psum.tile([P, 2, F], f32, tag="SR2", name=f"SR2_{t}")  # S_r | S_i
        SQ2 = psum.tile([P, 2, F], f32, tag="SQ2", name=f"SQ2_{t}")  # S_rr | S_ii
        SRI = psum.tile([P, F], f32, tag="SRI", name=f"SRI_{t}")     # S_ri

        for g in range(NB // G):
            bs = slice(g * G, (g + 1) * G)
            SQ = prodp.tile([P, G, F], f16, tag="SQ", name=f"SQ{t}_{g}")
            nc.scalar.square(SQ[:], XRI[:, bs, 0, :])
            SQI = prodp.tile([P, G, F], f16, tag="SQI", name=f"SQI{t}_{g}")
            nc.scalar.square(SQI[:], XRI[:, bs, 1, :])
            PG = prodp.tile([P, G, F], f16, tag="PG", name=f"PG{t}_{g}")
            nc.gpsimd.tensor_mul(PG[:], XRI[:, bs, 0, :], XRI[:, bs, 1, :])
            for q in range(G):
                b = g * G + q
                st, sp = (b == 0), (b == NB - 1)
                mm = nc.tensor.matmul
                mm(SR2[:, 0, :], identb[:], XRI[:, b, 0, :], start=st, stop=sp)
                mm(SR2[:, 1, :], identb[:], XRI[:, b, 1, :], start=st, stop=sp)
                mm(SQ2[:, 0, :], identb[:], SQ[:, q, :], start=st, stop=sp)
                mm(SQ2[:, 1, :], identb[:], SQI[:, q, :], start=st, stop=sp)
                mm(SRI[:], identb[:], PG[:, q, :], start=st, stop=sp)

        # ---- coefficient phase ----
        ct = lambda tag, shape, dt_=f16: cpool.tile(
            [P] + shape, dt_, tag=tag, name=f"{tag}{t}"
        )
        MU2 = ct("MU2", [2, F])   # mu_r | mu_i
        nc.scalar.activation(MU2[:], SR2[:], ACTF.Copy, scale=inv16)
        MSQ = ct("MSQ", [2, F])   # mu_r^2 | mu_i^2
        nc.scalar.activation(MSQ[:], SR2[:], ACTF.Square, scale=inv16)
        VV = ct("VV", [2, F])     # Vrr | Vii  (without +eps)
        nc.vector.scalar_tensor_tensor(
            VV[:], SQ2[:], inv16, MSQ[:], ALU.mult, ALU.subtract
        )
        MRI = ct("MRI", [F])      # mu_r * mu_i
        nc.vector.tensor_mul(MRI[:], MU2[:, 0, :], MU2[:, 1, :])
        VRI = ct("VRI", [F])
        nc.vector.scalar_tensor_tensor(
            VRI[:], SRI[:], inv16, MRI[:], ALU.mult, ALU.subtract
        )
        TAU = ct("TAU", [F])
        nc.vector.tensor_add(TAU[:], VV[:, 0, :], VV[:, 1, :])
        QD = ct("QD", [F])
        nc.vector.tensor_mul(QD[:], VV[:, 0, :], VV[:, 1, :])
        VRI2 = ct("VRI2", [F])
        nc.scalar.square(VRI2[:], VRI[:])
        DET0 = ct("DET0", [F])
        nc.vector.tensor_sub(DET0[:], QD[:], VRI2[:])
        DET = ct("DET", [F])      # det = QD - VRI2 + eps*tau  (+eps^2 ~ 0)
        nc.vector.scalar_tensor_tensor(
            DET[:], TAU[:], EPS, DET0[:], ALU.mult, ALU.add
        )
        SS = ct("SS", [F])
        nc.scalar.sqrt(SS[:], DET[:])
        T2 = ct("T2", [F])        # 2*s + tau
        nc.vector.scalar_tensor_tensor(
            T2[:], SS[:], 2.0, TAU[:], ALU.mult, ALU.add
        )
        TT = ct("TT", [F])        # t = sqrt(tau + 2s + 2eps)
        nc.scalar.activation(TT[:], T2[:], ACTF.Sqrt, bias=B2EPS[:])
        ST = ct("ST", [F], f32)
        nc.vector.tensor_mul(ST[:], SS[:], TT[:])
        INVF = ct("INVF", [F], f32)
        nc.vector.reciprocal_approx_fast(INVF[:], ST[:])
        INV = ct("INV", [F])
        nc.scalar.copy(INV[:], INVF[:])
        E12 = ct("E12", [2, F])   # (Vii + s) | (Vrr + s)
        nc.vector.tensor_add(E12[:, 0, :], VV[:, 1, :], SS[:])
        nc.vector.tensor_add(E12[:, 1, :], VV[:, 0, :], SS[:])
        W2 = ct("W2", [2, F])     # Wrr | Wii
        nc.vector.tensor_mul(W2[:], E12[:], bc(INV[:], 2))
        WRIP = ct("WRIP", [F])    # +Vri*inv (true Wri = -this)
        nc.vector.tensor_mul(WRIP[:], VRI[:], INV[:])

        # C4 = [[a1, a2], [b1, b2]] = G1 * (W2 repeated) - G2 * (WRIP rep4)
        MO = ct("MO", [2, 2, F])
        nc.vector.tensor_mul(MO[:], G1[:], bc(W2[:], 2, axis=1))
        ME = ct("ME", [2, 2, F])
        nc.vector.tensor_mul(ME[:], G2[:], bc(bc(WRIP[:], 2), 2))
        C4 = c2pool.tile([P, 2, 2, F], f16, tag="C4", name=f"C4_{t}")
        nc.vector.tensor_sub(C4[:], MO[:], ME[:])

        # AB0 = a0 | b0 = beta - (a1|b1)*mu_r - (a2|b2)*mu_i
        P1 = ct("P1", [2, F])
        nc.vector.tensor_mul(P1[:], C4[:, :, 0, :], bc(MU2[:, 0, :], 2))
        P2 = ct("P2", [2, F])
        nc.vector.tensor_mul(P2[:], C4[:, :, 1, :], bc(MU2[:, 1, :], 2))
        D0 = ct("D0", [2, F])
        nc.vector.tensor_sub(D0[:], bc(BET[:], 2), P1[:])
        AB0 = c2pool.tile([P, 2, F], f16, tag="AB0", name=f"AB0_{t}")
        nc.vector.tensor_sub(AB0[:], D0[:], P2[:])

        # ---- pass 2 ----
        for h in range(NB // G):
            bs = slice(h * G, (h + 1) * G)
            XRIh = XRI[:, bs, :, :]
            Wr = wpool.tile([P, G, 2, F], f16, tag="Wr", name=f"Wr{t}_{h}")
            nc.vector.tensor_mul(Wr[:], XRIh, bc(C4[:, 0], G, axis=1))
            Wi = wpool.tile([P, G, 2, F], f16, tag="Wi", name=f"Wi{t}_{h}")
            nc.vector.tensor_mul(Wi[:], XRIh, bc(C4[:, 1], G, axis=1))
            O2 = opool.tile([P, G, 2, F], f16, tag="O2", name=f"O2_{t}_{h}")
            nc.vector.tensor_add(O2[:, :, 0, :], Wr[:, :, 0, :], Wr[:, :, 1, :])
            nc.vector.tensor_add(O2[:, :, 1, :], Wi[:, :, 0, :], Wi[:, :, 1, :])
            nc.vector.tensor_add(O2[:], O2[:], bc(AB0[:], G, axis=1))
            nc.sync.dma_start(outr_v[t][:, bs, :], O2[:, :, 0, :])
            nc.sync.dma_start(outi_v[t][:, bs, :], O2[:, :, 1, :])


def build_nc() -> bacc.Bacc:
    nc = bacc.Bacc("TRN2", target_bir_lowering=False, debug=False)
    with tile.TileContext(nc) as tc:
        with ExitStack() as ctx:
            _emit(nc, ctx, tc)
    nc.compile()
    return nc


_cache: dict = {}


def _get_nc(npos: int = NPOS_FULL) -> bacc.Bacc:
    key = npos
    if key not in _cache:
        _cache[key] = build_nc()
    return _cache[key]


def make_in_maps(x_real, x_imag, gamma_rr, gamma_ri, gamma_ii, beta):
    """Shard channels across cores; pre-cast to fp16 on host."""
    in_maps = []
    for k in range(N_CORES):
        sl = slice(k * C_LOC, (k + 1) * C_LOC)
        in_maps.append(
            {
                "xr": np.ascontiguousarray(x_real[:, sl]).reshape(NB, -1).astype(np.float16),
                "xi": np.ascontiguousarray(x_imag[:, sl]).reshape(NB, -1).astype(np.float16),
                "grr": np.ascontiguousarray(gamma_rr[sl]).reshape(-1).astype(np.float16),
                "gri": np.ascontiguousarray(gamma_ri[sl]).reshape(-1).astype(np.float16),
                "gii": np.ascontiguousarray(gamma_ii[sl]).reshape(-1).astype(np.float16),
                "bet": np.ascontiguousarray(beta[sl]).reshape(-1).astype(np.float16),
            }
        )
    return in_maps


def assemble_output(results) -> np.ndarray:
    """Combine per-core planar fp16 real/imag outputs into full complex64."""
    out = np.empty((NB, C_FULL, HW), dtype=np.complex64)
    for k in range(N_CORES):
        o_r = np.asarray(results[k]["outr"]).astype(np.float32)
        o_i = np.asarray(results[k]["outi"]).astype(np.float32)
        out[:, k * C_LOC : (k + 1) * C_LOC] = (o_r + 1j * o_i).reshape(
            NB, C_LOC, HW
        )
    return out.reshape(NB, C_FULL, 256, 256)


def kernel(x_real, x_imag, gamma_rr, gamma_ri, gamma_ii, beta) -> np.ndarray:
    x_real = np.asarray(x_real, dtype=np.float32)
    x_imag = np.asarray(x_imag, dtype=np.float32)
    gamma_rr = np.asarray(gamma_rr, dtype=np.float32)
    gamma_ri = np.asarray(gamma_ri, dtype=np.float32)
    gamma_ii = np.asarray(gamma_ii, dtype=np.float32)
    beta = np.asarray(beta, dtype=np.float32)

    nc = _get_nc(NPOS_FULL)
    in_maps = make_in_maps(x_real, x_imag, gamma_rr, gamma_ri, gamma_ii, beta)
    res = run_bass_kernel_spmd(nc, in_maps, core_ids=list(range(N_CORES)))
    return assemble_output(res.results)


# revision 22
# speedup vs baseline: 1.0867x; 1.0797x over previous
"""Complex batch-norm Trainium2 kernel (nn_ComplexBatchNormal), v2.

Full inputs: x_real/x_imag [16, 32, 256, 256] f32, params [32, 256, 256] f32.
Output: complex64 [16, 32, 256, 256].

Sharding: C=32 channels split over 8 cores (4 each) -> fully local stats.

Device I/O is fp16 (host pre-casts inputs, combines planar fp16 outputs into
complex64; rel-err budget 2e-2 >> fp16 rounding). Per-core HBM traffic is
~38 MB vs ~71 MB for the f32 baseline.

Per tile (positions P*F = 65536, batch 16), engine split:
  ACT    : xr^2, xi^2 (fp16), plus the 1-input coef ops (mu, mu^2, sqrt, cast)
  GpSimd : xr*xi product
  TensorE: all 5 batch-sum stats via fp16 identity-matmul accumulation into
           paired PSUM banks (S_r|S_i, S_rr|S_ii, S_ri)
  DVE    : fused coefficient chain on [P,2,F]/[P,2,2,F] concatenated tiles
           (analytic inverse sqrt of the 2x2 covariance folded with gamma/beta
           into a1,a2,b1,b2,a0,b0), then pass-2 out = a1*xr + a2*xi + a0 (and
           b-side) as broadcast-AP fp16 tensor ops writing fp16 planar outputs.
"""

import sys

if "/opt/trn_rl_repo" not in sys.path:
    sys.path.insert(0, "/opt/trn_rl_repo")

from contextlib import ExitStack

import numpy as np

import concourse.bacc as bacc
import concourse.bass as bass
import concourse.tile as tile
from concourse import masks, mybir
from concourse.bass_utils import run_bass_kernel_spmd

P = 128          # SBUF partitions
F = 512          # free-dim positions per stat tile (= one PSUM bank of f32)
NB = 16          # batch size
NT = 4           # position tiles per core
EPS = 1e-5
N_CORES = 8
C_FULL = 32
C_LOC = C_FULL // N_CORES  # 4 channels per core
HW = 256 * 256
NPOS_FULL = NT * P * F     # 262144 positions per core
G = 4                      # batch group for products/stats
GH = 2                     # batch group for pass-2 chunks

f32 = mybir.dt.float32
f16 = mybir.dt.float16
ALU = mybir.AluOpType
ACTF = mybir.ActivationFunctionType


def bc(ap: bass.AP, n: int, axis: int = 1) -> bass.AP:
    """Insert a step-0 (broadcast) free dim of size n at free-dim `axis`."""
    dims = [list(x) for x in ap.ap]
    return bass.AP(
        tensor=ap.tensor,
        offset=ap.offset,
        ap=dims[:axis] + [[0, n]] + dims[axis:],
    )


def _finish_imag(nc, psum, identb, outi_v, item):
    """Imag-side finish for one pass-2 chunk: per output column, U2 + V2 + b0
    as 3 accumulating identity matmuls into a PSUM bank, ACT-evicted to fp16,
    then DMA'd out."""
    t, h, Wi, O2I, AB0 = item
    for q in range(GH):
        FIN = psum.tile([P, F], f32, tag="FIN", name=f"FIN{t}_{h}_{q}", bufs=3)
        mm = nc.tensor.matmul
        mm(FIN[:], identb[:], Wi[:, q, 0, :], start=True, stop=False)
        mm(FIN[:], identb[:], Wi[:, q, 1, :], start=False, stop=False)
        mm(FIN[:], identb[:], AB0[:, 1, :], start=False, stop=True)
        nc.scalar.copy(O2I[:, q, :], FIN[:])
    bs = slice(h * GH, (h + 1) * GH)
    nc.sync.dma_start(outi_v[t][:, bs, :], O2I[:])


def _emit(nc: bacc.Bacc, ctx: ExitStack, tc: "tile.TileContext"):
    xr_d = nc.dram_tensor("xr", [NB, NPOS_FULL], f16, kind="ExternalInput")
    xi_d = nc.dram_tensor("xi", [NB, NPOS_FULL], f16, kind="ExternalInput")
    grr_d = nc.dram_tensor("grr", [NPOS_FULL], f16, kind="ExternalInput")
    gri_d = nc.dram_tensor("gri", [NPOS_FULL], f16, kind="ExternalInput")
    gii_d = nc.dram_tensor("gii", [NPOS_FULL], f16, kind="ExternalInput")
    bet_d = nc.dram_tensor("bet", [NPOS_FULL], f16, kind="ExternalInput")
    outr_d = nc.dram_tensor("outr", [NB, NPOS_FULL], f16, kind="ExternalOutput")
    outi_d = nc.dram_tensor("outi", [NB, NPOS_FULL], f16, kind="ExternalOutput")

    xr_v = xr_d.ap().rearrange("b (t p f) -> t p b f", p=P, f=F)
    xi_v = xi_d.ap().rearrange("b (t p f) -> t p b f", p=P, f=F)
    pv = lambda d: d.ap().rearrange("(t p f) -> t p f", p=P, f=F)
    grr_v, gri_v, gii_v, bet_v = pv(grr_d), pv(gri_d), pv(gii_d), pv(bet_d)
    outr_v = outr_d.ap().rearrange("b (t p f) -> t p b f", p=P, f=F)
    outi_v = outi_d.ap().rearrange("b (t p f) -> t p b f", p=P, f=F)

    singles = ctx.enter_context(tc.tile_pool(name="singles", bufs=1))
    xpool = ctx.enter_context(tc.tile_pool(name="x", bufs=2))
    prodp = ctx.enter_context(tc.tile_pool(name="prod", bufs=2))
    gpool = ctx.enter_context(tc.tile_pool(name="g", bufs=2))
    cpool = ctx.enter_context(tc.tile_pool(name="coef", bufs=1))
    c2pool = ctx.enter_context(tc.tile_pool(name="coef2", bufs=2))
    wpool = ctx.enter_context(tc.tile_pool(name="w", bufs=2))
    opool = ctx.enter_context(tc.tile_pool(name="o", bufs=2))
    psum = ctx.enter_context(tc.tile_pool(name="ps", bufs=1, space="PSUM"))

    identf = singles.tile([P, P], f32)
    masks.make_identity(nc, identf[:])
    identb = singles.tile([P, P], f16)
    nc.scalar.copy(identb[:], identf[:])
    B2EPS = singles.tile([P, 1], f32)
    nc.vector.memset(B2EPS[:], 2.0 * EPS)

    inv16 = 1.0 / NB
    deferred = []  # imag-side finish work carried into the next tile

    for t in range(NT):
        # ---- load x (fp16), params ----
        XRI = xpool.tile([P, NB, 2, F], f16, tag="XRI", name=f"XRI{t}")
        nc.sync.dma_start(XRI[:, :, 0, :], xr_v[t])
        nc.sync.dma_start(XRI[:, :, 1, :], xi_v[t])

        # G1 = [[grr, gri], [gri, gii]]; G2 = [[gri, grr], [gii, gri]] is a
        # reversed-k (negative-stride) view of G1 -- no extra loads.
        G1 = gpool.tile([P, 2, 2, F], f16, tag="G1", name=f"G1_{t}")
        nc.sync.dma_start(G1[:, 0, 0, :], grr_v[t])
        nc.sync.dma_start(G1[:, 0, 1, :], gri_v[t])
        nc.sync.dma_start(G1[:, 1, 0, :], gri_v[t])
        nc.sync.dma_start(G1[:, 1, 1, :], gii_v[t])
        g1k1 = G1[:, :, 1, :]
        g2view = bass.AP(
            tensor=g1k1.tensor,
            offset=g1k1.offset,
            ap=[list(g1k1.ap[0]), list(g1k1.ap[1]), [-F, 2], list(g1k1.ap[2])],
        )
        BET = gpool.tile([P, F], f16, tag="BET", name=f"BET{t}")
        nc.sync.dma_start(BET[:], bet_v[t])

        # ---- pass 1: stats. products on ACT/GpSimd, sums on TensorE ----
        SR2 = psum.tile([P, 2, F], f32, tag="SR2", name=f"SR2_{t}")  # S_r | S_i
        SQ2 = psum.tile([P, 2, F], f32, tag="SQ2", name=f"SQ2_{t}")  # S_rr | S_ii
        SRI = psum.tile([P, F], f32, tag="SRI", name=f"SRI_{t}")     # S_ri

        for g in range(NB // G):
            bs = slice(g * G, (g + 1) * G)
            SQ = prodp.tile([P, G, F], f16, tag="SQ", name=f"SQ{t}_{g}")
            nc.scalar.square(SQ[:], XRI[:, bs, 0, :])
            SQI = prodp.tile([P, G, F], f16, tag="SQI", name=f"SQI{t}_{g}")
            nc.scalar.square(SQI[:], XRI[:, bs, 1, :])
            PG = prodp.tile([P, G, F], f16, tag="PG", name=f"PG{t}_{g}")
            nc.gpsimd.tensor_mul(PG[:], XRI[:, bs, 0, :], XRI[:, bs, 1, :])
            for q in range(G):
                b = g * G + q
                st, sp = (b == 0), (b == NB - 1)
                mm = nc.tensor.matmul
                mm(SR2[:, 0, :], identb[:], XRI[:, b, 0, :], start=st, stop=sp)
                mm(SR2[:, 1, :], identb[:], XRI[:, b, 1, :], start=st, stop=sp)
                mm(SQ2[:, 0, :], identb[:], SQ[:, q, :], start=st, stop=sp)
                mm(SQ2[:, 1, :], identb[:], SQI[:, q, :], start=st, stop=sp)
                mm(SRI[:], identb[:], PG[:, q, :], start=st, stop=sp)

        # deferred imag finishes from the previous tile fill the TensorE
        # bubble while DVE runs this tile's coefficient chain
        for item in deferred:
            _finish_imag(nc, psum, identb, outi_v, item)
        deferred = []

        # ---- coefficient phase ----
        ct = lambda tag, shape, dt_=f16: cpool.tile(
            [P] + shape, dt_, tag=tag, name=f"{tag}{t}"
        )
        MU2 = ct("MU2", [2, F])   # mu_r | mu_i
        nc.scalar.activation(MU2[:], SR2[:], ACTF.Copy, scale=inv16)
        MSQ = ct("MSQ", [2, F])   # mu_r^2 | mu_i^2
        nc.scalar.activation(MSQ[:], SR2[:], ACTF.Square, scale=inv16)
        VV = ct("VV", [2, F])     # Vrr | Vii  (without +eps)
        nc.vector.scalar_tensor_tensor(
            VV[:], SQ2[:], inv16, MSQ[:], ALU.mult, ALU.subtract
        )
        MRI = ct("MRI", [F])      # mu_r * mu_i   (gpsimd: off the DVE)
        nc.gpsimd.tensor_mul(MRI[:], MU2[:, 0, :], MU2[:, 1, :])
        VRI = ct("VRI", [F])
        nc.vector.scalar_tensor_tensor(
            VRI[:], SRI[:], inv16, MRI[:], ALU.mult, ALU.subtract
        )
        TAU = ct("TAU", [F])
        nc.gpsimd.tensor_add(TAU[:], VV[:, 0, :], VV[:, 1, :])
        QD = ct("QD", [F])
        nc.gpsimd.tensor_mul(QD[:], VV[:, 0, :], VV[:, 1, :])
        VRI2 = ct("VRI2", [F])
        nc.scalar.square(VRI2[:], VRI[:])
        DET0 = ct("DET0", [F])
        nc.gpsimd.tensor_sub(DET0[:], QD[:], VRI2[:])
        DET = ct("DET", [F])      # det = QD - VRI2 + eps*tau  (+eps^2 ~ 0)
        nc.vector.scalar_tensor_tensor(
            DET[:], TAU[:], EPS, DET0[:], ALU.mult, ALU.add
        )
        SS = ct("SS", [F])
        nc.scalar.sqrt(SS[:], DET[:])
        T2 = ct("T2", [F])        # 2*s + tau
        nc.vector.scalar_tensor_tensor(
            T2[:], SS[:], 2.0, TAU[:], ALU.mult, ALU.add
        )
        TT = ct("TT", [F])        # t = sqrt(tau + 2s + 2eps)
        nc.scalar.activation(TT[:], T2[:], ACTF.Sqrt, bias=B2EPS[:])
        ST = ct("ST", [F], f32)
        nc.vector.tensor_mul(ST[:], SS[:], TT[:])
        INVF = ct("INVF", [F], f32)
        nc.vector.reciprocal_approx_fast(INVF[:], ST[:])
        INV = ct("INV", [F])
        nc.scalar.copy(INV[:], INVF[:])
        E12 = ct("E12", [2, F])   # (Vii + s) | (Vrr + s)
        nc.vector.tensor_add(E12[:, 0, :], VV[:, 1, :], SS[:])
        nc.vector.tensor_add(E12[:, 1, :], VV[:, 0, :], SS[:])
        W2 = ct("W2", [2, F])     # Wrr | Wii
        nc.vector.tensor_mul(W2[:], E12[:], bc(INV[:], 2))
        WRIP = ct("WRIP", [F])    # +Vri*inv (true Wri = -this)
        nc.vector.tensor_mul(WRIP[:], VRI[:], INV[:])

        # C4 = [[a1, a2], [b1, b2]] = G1 * (W2 repeated) - G2 * (WRIP rep4)
        MO = ct("MO", [2, 2, F])
        nc.vector.tensor_mul(MO[:], G1[:], bc(W2[:], 2, axis=1))
        ME = ct("ME", [2, 2, F])
        nc.vector.tensor_mul(ME[:], g2view, bc(bc(WRIP[:], 2), 2))
        C4 = c2pool.tile([P, 2, 2, F], f16, tag="C4", name=f"C4_{t}")
        nc.vector.tensor_sub(C4[:], MO[:], ME[:])

        # AB0 = a0 | b0 = beta - (a1|b1)*mu_r - (a2|b2)*mu_i
        P1 = ct("P1", [2, F])
        nc.vector.tensor_mul(P1[:], C4[:, :, 0, :], bc(MU2[:, 0, :], 2))
        P2 = ct("P2", [2, F])
        nc.vector.tensor_mul(P2[:], C4[:, :, 1, :], bc(MU2[:, 1, :], 2))
        D0 = ct("D0", [2, F])
        nc.vector.tensor_sub(D0[:], bc(BET[:], 2), P1[:])
        AB0 = c2pool.tile([P, 2, F], f16, tag="AB0", name=f"AB0_{t}", bufs=3)
        nc.vector.tensor_sub(AB0[:], D0[:], P2[:])

        # ---- pass 2 ----
        # DVE computes the 4 products (W) and finishes the REAL component;
        # the IMAG component's add-chain (U2+V2+b0) is routed through the
        # TensorEngine + ACT eviction (_finish_imag). The last two chunks'
        # finishes are deferred into the next tile so TensorE can start that
        # tile's stats without waiting on this tile's late DVE products.
        for h in range(NB // GH):
            bs = slice(h * GH, (h + 1) * GH)
            XRIh = XRI[:, bs, :, :]
            Wr = wpool.tile([P, GH, 2, F], f16, tag="Wr", name=f"Wr{t}_{h}", bufs=2)
            nc.vector.tensor_mul(Wr[:], XRIh, bc(C4[:, 0], GH, axis=1))
            Wi = wpool.tile([P, GH, 2, F], f16, tag="Wi", name=f"Wi{t}_{h}", bufs=6)
            nc.vector.tensor_mul(Wi[:], XRIh, bc(C4[:, 1], GH, axis=1))
            O2R = opool.tile([P, GH, F], f16, tag="O2R", name=f"O2R_{t}_{h}", bufs=2)
            nc.vector.tensor_add(O2R[:], Wr[:, :, 0, :], Wr[:, :, 1, :])
            nc.vector.tensor_add(O2R[:], O2R[:], bc(AB0[:, 0, :], GH, axis=1))
            nc.sync.dma_start(outr_v[t][:, bs, :], O2R[:])
            O2I = opool.tile([P, GH, F], f16, tag="O2I", name=f"O2I_{t}_{h}", bufs=6)
            item = (t, h, Wi, O2I, AB0)
            if h < NB // GH - 4:
                _finish_imag(nc, psum, identb, outi_v, item)
            else:
                deferred.append(item)
    for item in deferred:
        _finish_imag(nc, psum, identb, outi_v, item)


def build_nc() -> bacc.Bacc:
    nc = bacc.Bacc("TRN2", target_bir_lowering=False, debug=False)
    with tile.TileContext(nc) as tc:
        with ExitStack() as ctx:
            _emit(nc, ctx, tc)
    nc.compile()
    return nc


_cache: dict = {}


def _get_nc(npos: int = NPOS_FULL) -> bacc.Bacc:
    key = npos
    if key not in _cache:
        _cache[key] = build_nc()
    return _cache[key]


def make_in_maps(x_real, x_imag, gamma_rr, gamma_ri, gamma_ii, beta):
    """Shard channels across cores; pre-cast to fp16 on host."""
    in_maps = []
    for k in range(N_CORES):
        sl = slice(k * C_LOC, (k + 1) * C_LOC)
        in_maps.append(
            {
                "xr": np.ascontiguousarray(x_real[:, sl]).reshape(NB, -1).astype(np.float16),
                "xi": np.ascontiguousarray(x_imag[:, sl]).reshape(NB, -1).astype(np.float16),
                "grr": np.ascontiguousarray(gamma_rr[sl]).reshape(-1).astype(np.float16),
                "gri": np.ascontiguousarray(gamma_ri[sl]).reshape(-1).astype(np.float16),
                "gii": np.ascontiguousarray(gamma_ii[sl]).reshape(-1).astype(np.float16),
                "bet": np.ascontiguousarray(beta[sl]).reshape(-1).astype(np.float16),
            }
        )
    return in_maps


def assemble_output(results) -> np.ndarray:
    """Combine per-core planar fp16 real/imag outputs into full complex64."""
    out = np.empty((NB, C_FULL, HW), dtype=np.complex64)
    for k in range(N_CORES):
        o_r = np.asarray(results[k]["outr"]).astype(np.float32)
        o_i = np.asarray(results[k]["outi"]).astype(np.float32)
        out[:, k * C_LOC : (k + 1) * C_LOC] = (o_r + 1j * o_i).reshape(
            NB, C_LOC, HW
        )
    return out.reshape(NB, C_FULL, 256, 256)


def kernel(x_real, x_imag, gamma_rr, gamma_ri, gamma_ii, beta) -> np.ndarray:
    x_real = np.asarray(x_real, dtype=np.float32)
    x_imag = np.asarray(x_imag, dtype=np.float32)
    gamma_rr = np.asarray(gamma_rr, dtype=np.float32)
    gamma_ri = np.asarray(gamma_ri, dtype=np.float32)
    gamma_ii = np.asarray(gamma_ii, dtype=np.float32)
    beta = np.asarray(beta, dtype=np.float32)

    nc = _get_nc(NPOS_FULL)
    in_maps = make_in_maps(x_real, x_imag, gamma_rr, gamma_ri, gamma_ii, beta)
    res = run_bass_kernel_spmd(nc, in_maps, core_ids=list(range(N_CORES)))
    return assemble_output(res.results)


# revision 25
# speedup vs baseline: 1.1620x; 1.0693x over previous
"""Complex batch-norm Trainium2 kernel (nn_ComplexBatchNormal), v2.

Full inputs: x_real/x_imag [16, 32, 256, 256] f32, params [32, 256, 256] f32.
Output: complex64 [16, 32, 256, 256].

Sharding: C=32 channels split over 8 cores (4 each) -> fully local stats.

Device I/O is fp16 (host pre-casts inputs, combines planar fp16 outputs into
complex64; rel-err budget 2e-2 >> fp16 rounding). Per-core HBM traffic is
~38 MB vs ~71 MB for the f32 baseline.

Per tile (positions P*F = 65536, batch 16), engine split:
  ACT    : xr^2, xi^2 (fp16), plus the 1-input coef ops (mu, mu^2, sqrt, cast)
  GpSimd : xr*xi product
  TensorE: all 5 batch-sum stats via fp16 identity-matmul accumulation into
           paired PSUM banks (S_r|S_i, S_rr|S_ii, S_ri)
  DVE    : fused coefficient chain on [P,2,F]/[P,2,2,F] concatenated tiles
           (analytic inverse sqrt of the 2x2 covariance folded with gamma/beta
           into a1,a2,b1,b2,a0,b0), then pass-2 out = a1*xr + a2*xi + a0 (and
           b-side) as broadcast-AP fp16 tensor ops writing fp16 planar outputs.
"""

import sys

if "/opt/trn_rl_repo" not in sys.path:
    sys.path.insert(0, "/opt/trn_rl_repo")

from contextlib import ExitStack

import numpy as np

import concourse.bacc as bacc
import concourse.bass as bass
import concourse.tile as tile
from concourse import masks, mybir
from concourse.bass_utils import run_bass_kernel_spmd

P = 128          # SBUF partitions
F = 512          # free-dim positions per stat tile (= one PSUM bank of f32)
NB = 16          # batch size
NT = 4           # position tiles per core
EPS = 1e-5
N_CORES = 8
C_FULL = 32
C_LOC = C_FULL // N_CORES  # 4 channels per core
HW = 256 * 256
NPOS_FULL = NT * P * F     # 262144 positions per core
G = 4                      # batch group for products/stats
GH = 2                     # batch group for pass-2 chunks

f32 = mybir.dt.float32
f16 = mybir.dt.float16
ALU = mybir.AluOpType
ACTF = mybir.ActivationFunctionType


def bc(ap: bass.AP, n: int, axis: int = 1) -> bass.AP:
    """Insert a step-0 (broadcast) free dim of size n at free-dim `axis`."""
    dims = [list(x) for x in ap.ap]
    return bass.AP(
        tensor=ap.tensor,
        offset=ap.offset,
        ap=dims[:axis] + [[0, n]] + dims[axis:],
    )


def _finish_imag(nc, psum, identb, outi_v, item):
    """Imag-side finish for one pass-2 chunk: per output column, U2 + V2 + b0
    as 3 accumulating identity matmuls into a PSUM bank, ACT-evicted to fp16,
    then DMA'd out."""
    t, h, Wi, O2I, AB0 = item
    for q in range(GH):
        FIN = psum.tile([P, F], f32, tag="FIN", name=f"FIN{t}_{h}_{q}", bufs=3)
        mm = nc.tensor.matmul
        mm(FIN[:], identb[:], Wi[:, q, 0, :], start=True, stop=False)
        mm(FIN[:], identb[:], Wi[:, q, 1, :], start=False, stop=False)
        mm(FIN[:], identb[:], AB0[:, 1, :], start=False, stop=True)
        nc.scalar.copy(O2I[:, q, :], FIN[:])
    bs = slice(h * GH, (h + 1) * GH)
    nc.sync.dma_start(outi_v[t][:, bs, :], O2I[:])


def _emit(nc: bacc.Bacc, ctx: ExitStack, tc: "tile.TileContext"):
    xr_d = nc.dram_tensor("xr", [NB, NPOS_FULL], f16, kind="ExternalInput")
    xi_d = nc.dram_tensor("xi", [NB, NPOS_FULL], f16, kind="ExternalInput")
    grr_d = nc.dram_tensor("grr", [NPOS_FULL], f16, kind="ExternalInput")
    gri_d = nc.dram_tensor("gri", [NPOS_FULL], f16, kind="ExternalInput")
    gii_d = nc.dram_tensor("gii", [NPOS_FULL], f16, kind="ExternalInput")
    bet_d = nc.dram_tensor("bet", [NPOS_FULL], f16, kind="ExternalInput")
    outr_d = nc.dram_tensor("outr", [NB, NPOS_FULL], f16, kind="ExternalOutput")
    outi_d = nc.dram_tensor("outi", [NB, NPOS_FULL], f16, kind="ExternalOutput")

    xr_v = xr_d.ap().rearrange("b (t p f) -> t p b f", p=P, f=F)
    xi_v = xi_d.ap().rearrange("b (t p f) -> t p b f", p=P, f=F)
    pv = lambda d: d.ap().rearrange("(t p f) -> t p f", p=P, f=F)
    grr_v, gri_v, gii_v, bet_v = pv(grr_d), pv(gri_d), pv(gii_d), pv(bet_d)
    outr_v = outr_d.ap().rearrange("b (t p f) -> t p b f", p=P, f=F)
    outi_v = outi_d.ap().rearrange("b (t p f) -> t p b f", p=P, f=F)

    singles = ctx.enter_context(tc.tile_pool(name="singles", bufs=1))
    xpool = ctx.enter_context(tc.tile_pool(name="x", bufs=2))
    prodp = ctx.enter_context(tc.tile_pool(name="prod", bufs=2))
    gpool = ctx.enter_context(tc.tile_pool(name="g", bufs=2))
    cpool = ctx.enter_context(tc.tile_pool(name="coef", bufs=1))
    c2pool = ctx.enter_context(tc.tile_pool(name="coef2", bufs=2))
    wpool = ctx.enter_context(tc.tile_pool(name="w", bufs=2))
    opool = ctx.enter_context(tc.tile_pool(name="o", bufs=2))
    psum = ctx.enter_context(tc.tile_pool(name="ps", bufs=1, space="PSUM"))

    identf = singles.tile([P, P], f32)
    masks.make_identity(nc, identf[:])
    identb = singles.tile([P, P], f16)
    nc.scalar.copy(identb[:], identf[:])
    B2EPS = singles.tile([P, 1], f32)
    nc.vector.memset(B2EPS[:], 2.0 * EPS)

    inv16 = 1.0 / NB
    deferred = []  # imag-side finish work carried into the next tile

    for t in range(NT):
        # ---- load x (fp16), params ----
        XRI = xpool.tile([P, NB, 2, F], f16, tag="XRI", name=f"XRI{t}")
        # per-group loads so tile-0 products can start before the full load
        for g in range(NB // G):
            bsl = slice(g * G, (g + 1) * G)
            nc.sync.dma_start(XRI[:, bsl, 0, :], xr_v[t][:, bsl, :])
            nc.sync.dma_start(XRI[:, bsl, 1, :], xi_v[t][:, bsl, :])

        # G1 = [[grr, gri], [gri, gii]]; G2 = [[gri, grr], [gii, gri]] is a
        # reversed-k (negative-stride) view of G1 -- no extra loads.
        G1 = gpool.tile([P, 2, 2, F], f16, tag="G1", name=f"G1_{t}")
        nc.sync.dma_start(G1[:, 0, 0, :], grr_v[t])
        nc.sync.dma_start(G1[:, 0, 1, :], gri_v[t])
        nc.sync.dma_start(G1[:, 1, 0, :], gri_v[t])
        nc.sync.dma_start(G1[:, 1, 1, :], gii_v[t])
        g1k1 = G1[:, :, 1, :]
        g2view = bass.AP(
            tensor=g1k1.tensor,
            offset=g1k1.offset,
            ap=[list(g1k1.ap[0]), list(g1k1.ap[1]), [-F, 2], list(g1k1.ap[2])],
        )
        BET = gpool.tile([P, F], f16, tag="BET", name=f"BET{t}")
        nc.sync.dma_start(BET[:], bet_v[t])

        # ---- pass 1: stats. products on ACT/GpSimd, sums on TensorE ----
        SR2 = psum.tile([P, 2, F], f32, tag="SR2", name=f"SR2_{t}")  # S_r | S_i
        SQ2 = psum.tile([P, 2, F], f32, tag="SQ2", name=f"SQ2_{t}")  # S_rr | S_ii
        SRI = psum.tile([P, F], f32, tag="SRI", name=f"SRI_{t}")     # S_ri

        for g in range(NB // G):
            bs = slice(g * G, (g + 1) * G)
            SQ = prodp.tile([P, G, F], f16, tag="SQ", name=f"SQ{t}_{g}")
            nc.scalar.square(SQ[:], XRI[:, bs, 0, :])
            SQI = prodp.tile([P, G, F], f16, tag="SQI", name=f"SQI{t}_{g}")
            nc.scalar.square(SQI[:], XRI[:, bs, 1, :])
            PG = prodp.tile([P, G, F], f16, tag="PG", name=f"PG{t}_{g}")
            nc.gpsimd.tensor_mul(PG[:], XRI[:, bs, 0, :], XRI[:, bs, 1, :])
            for q in range(G):
                b = g * G + q
                st, sp = (b == 0), (b == NB - 1)
                mm = nc.tensor.matmul
                mm(SR2[:, 0, :], identb[:], XRI[:, b, 0, :], start=st, stop=sp)
                mm(SR2[:, 1, :], identb[:], XRI[:, b, 1, :], start=st, stop=sp)
                mm(SQ2[:, 0, :], identb[:], SQ[:, q, :], start=st, stop=sp)
                mm(SQ2[:, 1, :], identb[:], SQI[:, q, :], start=st, stop=sp)
                mm(SRI[:], identb[:], PG[:, q, :], start=st, stop=sp)

        # deferred imag finishes from the previous tile fill the TensorE
        # bubble while DVE runs this tile's coefficient chain
        for item in deferred:
            _finish_imag(nc, psum, identb, outi_v, item)
        deferred = []

        # ---- coefficient phase ----
        ct = lambda tag, shape, dt_=f16: cpool.tile(
            [P] + shape, dt_, tag=tag, name=f"{tag}{t}"
        )
        MU2 = ct("MU2", [2, F])   # mu_r | mu_i
        nc.scalar.activation(MU2[:], SR2[:], ACTF.Copy, scale=inv16)
        MSQ = ct("MSQ", [2, F])   # mu_r^2 | mu_i^2
        nc.scalar.activation(MSQ[:], SR2[:], ACTF.Square, scale=inv16)
        VV = ct("VV", [2, F])     # Vrr | Vii  (without +eps)
        nc.vector.scalar_tensor_tensor(
            VV[:], SQ2[:], inv16, MSQ[:], ALU.mult, ALU.subtract
        )
        MRI = ct("MRI", [F])      # mu_r * mu_i
        nc.vector.tensor_mul(MRI[:], MU2[:, 0, :], MU2[:, 1, :])
        VRI = ct("VRI", [F])
        nc.vector.scalar_tensor_tensor(
            VRI[:], SRI[:], inv16, MRI[:], ALU.mult, ALU.subtract
        )
        TAU = ct("TAU", [F])
        nc.vector.tensor_add(TAU[:], VV[:, 0, :], VV[:, 1, :])
        QD = ct("QD", [F])
        nc.vector.tensor_mul(QD[:], VV[:, 0, :], VV[:, 1, :])
        VRI2 = ct("VRI2", [F])
        nc.scalar.square(VRI2[:], VRI[:])
        DET0 = ct("DET0", [F])
        nc.vector.tensor_sub(DET0[:], QD[:], VRI2[:])
        DET = ct("DET", [F])      # det = QD - VRI2 + eps*tau  (+eps^2 ~ 0)
        nc.vector.scalar_tensor_tensor(
            DET[:], TAU[:], EPS, DET0[:], ALU.mult, ALU.add
        )
        SS = ct("SS", [F])
        nc.scalar.sqrt(SS[:], DET[:])
        T2 = ct("T2", [F])        # 2*s + tau
        nc.vector.scalar_tensor_tensor(
            T2[:], SS[:], 2.0, TAU[:], ALU.mult, ALU.add
        )
        TT = ct("TT", [F])        # t = sqrt(tau + 2s + 2eps)
        nc.scalar.activation(TT[:], T2[:], ACTF.Sqrt, bias=B2EPS[:])
        ST = ct("ST", [F], f32)
        nc.vector.tensor_mul(ST[:], SS[:], TT[:])
        INVF = ct("INVF", [F], f32)
        nc.vector.reciprocal_approx_fast(INVF[:], ST[:])
        INV = ct("INV", [F])
        nc.scalar.copy(INV[:], INVF[:])
        E12 = ct("E12", [2, F])   # (Vii + s) | (Vrr + s)
        nc.vector.tensor_add(E12[:, 0, :], VV[:, 1, :], SS[:])
        nc.vector.tensor_add(E12[:, 1, :], VV[:, 0, :], SS[:])
        W2 = ct("W2", [2, F])     # Wrr | Wii
        nc.vector.tensor_mul(W2[:], E12[:], bc(INV[:], 2))
        WRIP = ct("WRIP", [F])    # +Vri*inv (true Wri = -this)
        nc.vector.tensor_mul(WRIP[:], VRI[:], INV[:])

        # C4 = [[a1, a2], [b1, b2]] = G1 * (W2 repeated) - G2 * (WRIP rep4)
        MO = ct("MO", [2, 2, F])
        nc.vector.tensor_mul(MO[:], G1[:], bc(W2[:], 2, axis=1))
        ME = ct("ME", [2, 2, F])
        nc.vector.tensor_mul(ME[:], g2view, bc(bc(WRIP[:], 2), 2))
        C4 = c2pool.tile([P, 2, 2, F], f16, tag="C4", name=f"C4_{t}")
        nc.vector.tensor_sub(C4[:], MO[:], ME[:])

        # AB0 = a0 | b0 = beta - (a1|b1)*mu_r - (a2|b2)*mu_i
        P1 = ct("P1", [2, F])
        nc.vector.tensor_mul(P1[:], C4[:, :, 0, :], bc(MU2[:, 0, :], 2))
        P2 = ct("P2", [2, F])
        nc.vector.tensor_mul(P2[:], C4[:, :, 1, :], bc(MU2[:, 1, :], 2))
        D0 = ct("D0", [2, F])
        nc.vector.tensor_sub(D0[:], bc(BET[:], 2), P1[:])
        AB0 = c2pool.tile([P, 2, F], f16, tag="AB0", name=f"AB0_{t}", bufs=3)
        nc.vector.tensor_sub(AB0[:], D0[:], P2[:])

        # ---- pass 2 ----
        # DVE computes the 4 products (W) and finishes the REAL component;
        # the IMAG component's add-chain (U2+V2+b0) is routed through the
        # TensorEngine + ACT eviction (_finish_imag). The last two chunks'
        # finishes are deferred into the next tile so TensorE can start that
        # tile's stats without waiting on this tile's late DVE products.
        for h in range(NB // GH):
            bs = slice(h * GH, (h + 1) * GH)
            XRIh = XRI[:, bs, :, :]
            Wr = wpool.tile([P, GH, 2, F], f16, tag="Wr", name=f"Wr{t}_{h}", bufs=2)
            nc.vector.tensor_mul(Wr[:], XRIh, bc(C4[:, 0], GH, axis=1))
            Wi = wpool.tile([P, GH, 2, F], f16, tag="Wi", name=f"Wi{t}_{h}", bufs=6)
            nc.vector.tensor_mul(Wi[:], XRIh, bc(C4[:, 1], GH, axis=1))
            O2R = opool.tile([P, GH, F], f16, tag="O2R", name=f"O2R_{t}_{h}", bufs=2)
            nc.vector.tensor_add(O2R[:], Wr[:, :, 0, :], Wr[:, :, 1, :])
            nc.vector.tensor_add(O2R[:], O2R[:], bc(AB0[:, 0, :], GH, axis=1))
            nc.sync.dma_start(outr_v[t][:, bs, :], O2R[:])
            O2I = opool.tile([P, GH, F], f16, tag="O2I", name=f"O2I_{t}_{h}", bufs=6)
            item = (t, h, Wi, O2I, AB0)
            if h < NB // GH - 4 or t == NT - 1:
                _finish_imag(nc, psum, identb, outi_v, item)
            else:
                deferred.append(item)
    for item in deferred:
        _finish_imag(nc, psum, identb, outi_v, item)


def build_nc() -> bacc.Bacc:
    nc = bacc.Bacc("TRN2", target_bir_lowering=False, debug=False)
    with tile.TileContext(nc) as tc:
        with ExitStack() as ctx:
            _emit(nc, ctx, tc)
    nc.compile()
    return nc


_cache: dict = {}


def _get_nc(npos: int = NPOS_FULL) -> bacc.Bacc:
    key = npos
    if key not in _cache:
        _cache[key] = build_nc()
    return _cache[key]


def make_in_maps(x_real, x_imag, gamma_rr, gamma_ri, gamma_ii, beta):
    """Shard channels across cores; pre-cast to fp16 on host."""
    in_maps = []
    for k in range(N_CORES):
        sl = slice(k * C_LOC, (k + 1) * C_LOC)
        in_maps.append(
            {
                "xr": np.ascontiguousarray(x_real[:, sl]).reshape(NB, -1).astype(np.float16),
                "xi": np.ascontiguousarray(x_imag[:, sl]).reshape(NB, -1).astype(np.float16),
                "grr": np.ascontiguousarray(gamma_rr[sl]).reshape(-1).astype(np.float16),
                "gri": np.ascontiguousarray(gamma_ri[sl]).reshape(-1).astype(np.float16),
                "gii": np.ascontiguousarray(gamma_ii[sl]).reshape(-1).astype(np.float16),
                "bet": np.ascontiguousarray(beta[sl]).reshape(-1).astype(np.float16),
            }
        )
    return in_maps


def assemble_output(results) -> np.ndarray:
    """Combine per-core planar fp16 real/imag outputs into full complex64."""
    out = np.empty((NB, C_FULL, HW), dtype=np.complex64)
    for k in range(N_CORES):
        o_r = np.asarray(results[k]["outr"]).astype(np.float32)
        o_i = np.asarray(results[k]["outi"]).astype(np.float32)
        out[:, k * C_LOC : (k + 1) * C_LOC] = (o_r + 1j * o_i).reshape(
            NB, C_LOC, HW
        )
    return out.reshape(NB, C_FULL, 256, 256)


def kernel(x_real, x_imag, gamma_rr, gamma_ri, gamma_ii, beta) -> np.ndarray:
    x_real = np.asarray(x_real, dtype=np.float32)
    x_imag = np.asarray(x_imag, dtype=np.float32)
    gamma_rr = np.asarray(gamma_rr, dtype=np.float32)
    gamma_ri = np.asarray(gamma_ri, dtype=np.float32)
    gamma_ii = np.asarray(gamma_ii, dtype=np.float32)
    beta = np.asarray(beta, dtype=np.float32)

    nc = _get_nc(NPOS_FULL)
    in_maps = make_in_maps(x_real, x_imag, gamma_rr, gamma_ri, gamma_ii, beta)
    res = run_bass_kernel_spmd(nc, in_maps, core_ids=list(range(N_CORES)))
    return assemble_output(res.results)


# revision 26
# speedup vs baseline: 1.1682x; 1.0054x over previous
"""Complex batch-norm Trainium2 kernel (nn_ComplexBatchNormal), v2.

Full inputs: x_real/x_imag [16, 32, 256, 256] f32, params [32, 256, 256] f32.
Output: complex64 [16, 32, 256, 256].

Sharding: C=32 channels split over 8 cores (4 each) -> fully local stats.

Device I/O is fp16 (host pre-casts inputs, combines planar fp16 outputs into
complex64; rel-err budget 2e-2 >> fp16 rounding). Per-core HBM traffic is
~38 MB vs ~71 MB for the f32 baseline.

Per tile (positions P*F = 65536, batch 16), engine split:
  ACT    : xr^2, xi^2 (fp16), plus the 1-input coef ops (mu, mu^2, sqrt, cast)
  GpSimd : xr*xi product
  TensorE: all 5 batch-sum stats via fp16 identity-matmul accumulation into
           paired PSUM banks (S_r|S_i, S_rr|S_ii, S_ri)
  DVE    : fused coefficient chain on [P,2,F]/[P,2,2,F] concatenated tiles
           (analytic inverse sqrt of the 2x2 covariance folded with gamma/beta
           into a1,a2,b1,b2,a0,b0), then pass-2 out = a1*xr + a2*xi + a0 (and
           b-side) as broadcast-AP fp16 tensor ops writing fp16 planar outputs.
"""

import sys

if "/opt/trn_rl_repo" not in sys.path:
    sys.path.insert(0, "/opt/trn_rl_repo")

from contextlib import ExitStack

import numpy as np

import concourse.bacc as bacc
import concourse.bass as bass
import concourse.tile as tile
from concourse import masks, mybir
from concourse.bass_utils import run_bass_kernel_spmd

P = 128          # SBUF partitions
F = 512          # free-dim positions per stat tile (= one PSUM bank of f32)
NB = 16          # batch size
NT = 4           # position tiles per core
EPS = 1e-5
N_CORES = 8
C_FULL = 32
C_LOC = C_FULL // N_CORES  # 4 channels per core
HW = 256 * 256
NPOS_FULL = NT * P * F     # 262144 positions per core
G = 4                      # batch group for products/stats
GH = 2                     # batch group for pass-2 chunks

f32 = mybir.dt.float32
f16 = mybir.dt.float16
ALU = mybir.AluOpType
ACTF = mybir.ActivationFunctionType


def bc(ap: bass.AP, n: int, axis: int = 1) -> bass.AP:
    """Insert a step-0 (broadcast) free dim of size n at free-dim `axis`."""
    dims = [list(x) for x in ap.ap]
    return bass.AP(
        tensor=ap.tensor,
        offset=ap.offset,
        ap=dims[:axis] + [[0, n]] + dims[axis:],
    )


def _finish_imag(nc, psum, identb, outi_v, item):
    """Imag-side finish for one pass-2 chunk: per output column, U2 + V2 + b0
    as 3 accumulating identity matmuls into a PSUM bank, ACT-evicted to fp16,
    then DMA'd out."""
    t, h, Wi, O2I, AB0 = item
    for q in range(GH):
        FIN = psum.tile([P, F], f32, tag="FIN", name=f"FIN{t}_{h}_{q}", bufs=3)
        mm = nc.tensor.matmul
        mm(FIN[:], identb[:], Wi[:, q, 0, :], start=True, stop=False)
        mm(FIN[:], identb[:], Wi[:, q, 1, :], start=False, stop=False)
        mm(FIN[:], identb[:], AB0[:, 1, :], start=False, stop=True)
        nc.scalar.copy(O2I[:, q, :], FIN[:])
    bs = slice(h * GH, (h + 1) * GH)
    nc.sync.dma_start(outi_v[t][:, bs, :], O2I[:])


def _emit(nc: bacc.Bacc, ctx: ExitStack, tc: "tile.TileContext"):
    xr_d = nc.dram_tensor("xr", [NB, NPOS_FULL], f16, kind="ExternalInput")
    xi_d = nc.dram_tensor("xi", [NB, NPOS_FULL], f16, kind="ExternalInput")
    grr_d = nc.dram_tensor("grr", [NPOS_FULL], f16, kind="ExternalInput")
    gri_d = nc.dram_tensor("gri", [NPOS_FULL], f16, kind="ExternalInput")
    gii_d = nc.dram_tensor("gii", [NPOS_FULL], f16, kind="ExternalInput")
    bet_d = nc.dram_tensor("bet", [NPOS_FULL], f16, kind="ExternalInput")
    outr_d = nc.dram_tensor("outr", [NB, NPOS_FULL], f16, kind="ExternalOutput")
    outi_d = nc.dram_tensor("outi", [NB, NPOS_FULL], f16, kind="ExternalOutput")

    xr_v = xr_d.ap().rearrange("b (t p f) -> t p b f", p=P, f=F)
    xi_v = xi_d.ap().rearrange("b (t p f) -> t p b f", p=P, f=F)
    pv = lambda d: d.ap().rearrange("(t p f) -> t p f", p=P, f=F)
    grr_v, gri_v, gii_v, bet_v = pv(grr_d), pv(gri_d), pv(gii_d), pv(bet_d)
    outr_v = outr_d.ap().rearrange("b (t p f) -> t p b f", p=P, f=F)
    outi_v = outi_d.ap().rearrange("b (t p f) -> t p b f", p=P, f=F)

    singles = ctx.enter_context(tc.tile_pool(name="singles", bufs=1))
    xpool = ctx.enter_context(tc.tile_pool(name="x", bufs=2))
    prodp = ctx.enter_context(tc.tile_pool(name="prod", bufs=2))
    gpool = ctx.enter_context(tc.tile_pool(name="g", bufs=2))
    cpool = ctx.enter_context(tc.tile_pool(name="coef", bufs=1))
    c2pool = ctx.enter_context(tc.tile_pool(name="coef2", bufs=2))
    wpool = ctx.enter_context(tc.tile_pool(name="w", bufs=2))
    opool = ctx.enter_context(tc.tile_pool(name="o", bufs=2))
    psum = ctx.enter_context(tc.tile_pool(name="ps", bufs=1, space="PSUM"))

    identf = singles.tile([P, P], f32)
    masks.make_identity(nc, identf[:])
    identb = singles.tile([P, P], f16)
    nc.scalar.copy(identb[:], identf[:])
    B2EPS = singles.tile([P, 1], f32)
    nc.vector.memset(B2EPS[:], 2.0 * EPS)

    inv16 = 1.0 / NB
    NH = NB // GH
    deferred_p2 = []  # pass-2 chunks dripped into the next tile's coef gaps

    def emit_chunk(item):
        """One pass-2 chunk: DVE muls + real finish, TensorE+ACT imag finish."""
        tt, h, XRIt, C4t, AB0t = item
        bs = slice(h * GH, (h + 1) * GH)
        XRIh = XRIt[:, bs, :, :]
        Wr = wpool.tile([P, GH, 2, F], f16, tag="Wr", name=f"Wr{tt}_{h}", bufs=2)
        nc.vector.tensor_mul(Wr[:], XRIh, bc(C4t[:, 0], GH, axis=1))
        Wi = wpool.tile([P, GH, 2, F], f16, tag="Wi", name=f"Wi{tt}_{h}", bufs=6)
        nc.vector.tensor_mul(Wi[:], XRIh, bc(C4t[:, 1], GH, axis=1))
        O2R = opool.tile([P, GH, F], f16, tag="O2R", name=f"O2R_{tt}_{h}", bufs=2)
        nc.vector.tensor_add(O2R[:], Wr[:, :, 0, :], Wr[:, :, 1, :])
        nc.vector.tensor_add(O2R[:], O2R[:], bc(AB0t[:, 0, :], GH, axis=1))
        nc.sync.dma_start(outr_v[tt][:, bs, :], O2R[:])
        O2I = opool.tile([P, GH, F], f16, tag="O2I", name=f"O2I_{tt}_{h}", bufs=6)
        _finish_imag(nc, psum, identb, outi_v, (tt, h, Wi, O2I, AB0t))

    for t in range(NT):
        # ---- load x (fp16), params ----
        XRI = xpool.tile([P, NB, 2, F], f16, tag="XRI", name=f"XRI{t}")
        # per-group loads so tile-0 products can start before the full load
        for g in range(NB // G):
            bsl = slice(g * G, (g + 1) * G)
            nc.sync.dma_start(XRI[:, bsl, 0, :], xr_v[t][:, bsl, :])
            nc.sync.dma_start(XRI[:, bsl, 1, :], xi_v[t][:, bsl, :])

        # G1 = [[grr, gri], [gri, gii]]; G2 = [[gri, grr], [gii, gri]] is a
        # reversed-k (negative-stride) view of G1 -- no extra loads.
        G1 = gpool.tile([P, 2, 2, F], f16, tag="G1", name=f"G1_{t}")
        nc.sync.dma_start(G1[:, 0, 0, :], grr_v[t])
        nc.sync.dma_start(G1[:, 0, 1, :], gri_v[t])
        nc.sync.dma_start(G1[:, 1, 0, :], gri_v[t])
        nc.sync.dma_start(G1[:, 1, 1, :], gii_v[t])
        g1k1 = G1[:, :, 1, :]
        g2view = bass.AP(
            tensor=g1k1.tensor,
            offset=g1k1.offset,
            ap=[list(g1k1.ap[0]), list(g1k1.ap[1]), [-F, 2], list(g1k1.ap[2])],
        )
        BET = gpool.tile([P, F], f16, tag="BET", name=f"BET{t}")
        nc.sync.dma_start(BET[:], bet_v[t])

        # ---- pass 1: stats. products on ACT/GpSimd, sums on TensorE ----
        SR2 = psum.tile([P, 2, F], f32, tag="SR2", name=f"SR2_{t}")  # S_r | S_i
        SQ2 = psum.tile([P, 2, F], f32, tag="SQ2", name=f"SQ2_{t}")  # S_rr | S_ii
        SRI = psum.tile([P, F], f32, tag="SRI", name=f"SRI_{t}")     # S_ri

        for g in range(NB // G):
            bsl = slice(g * G, (g + 1) * G)
            SQ = prodp.tile([P, G, F], f16, tag="SQ", name=f"SQ{t}_{g}")
            nc.scalar.square(SQ[:], XRI[:, bsl, 0, :])
            SQI = prodp.tile([P, G, F], f16, tag="SQI", name=f"SQI{t}_{g}")
            nc.scalar.square(SQI[:], XRI[:, bsl, 1, :])
            PG = prodp.tile([P, G, F], f16, tag="PG", name=f"PG{t}_{g}")
            nc.gpsimd.tensor_mul(PG[:], XRI[:, bsl, 0, :], XRI[:, bsl, 1, :])
            for q in range(G):
                b = g * G + q
                st, sp = (b == 0), (b == NB - 1)
                mm = nc.tensor.matmul
                mm(SR2[:, 0, :], identb[:], XRI[:, b, 0, :], start=st, stop=sp)
                mm(SR2[:, 1, :], identb[:], XRI[:, b, 1, :], start=st, stop=sp)
                mm(SQ2[:, 0, :], identb[:], SQ[:, q, :], start=st, stop=sp)
                mm(SQ2[:, 1, :], identb[:], SQI[:, q, :], start=st, stop=sp)
                mm(SRI[:], identb[:], PG[:, q, :], start=st, stop=sp)

        # ---- coefficient phase, with deferred pass-2 chunks dripped into
        # the serial chain's dependency-stall gaps (keeps DVE busy) ----
        drip = deferred_p2
        deferred_p2 = []

        def drip_one():
            if drip:
                emit_chunk(drip.pop(0))

        ct = lambda tag, shape, dt_=f16: cpool.tile(
            [P] + shape, dt_, tag=tag, name=f"{tag}{t}"
        )
        MU2 = ct("MU2", [2, F])   # mu_r | mu_i
        nc.scalar.activation(MU2[:], SR2[:], ACTF.Copy, scale=inv16)
        MSQ = ct("MSQ", [2, F])   # mu_r^2 | mu_i^2
        nc.scalar.activation(MSQ[:], SR2[:], ACTF.Square, scale=inv16)
        VV = ct("VV", [2, F])     # Vrr | Vii  (without +eps)
        nc.vector.scalar_tensor_tensor(
            VV[:], SQ2[:], inv16, MSQ[:], ALU.mult, ALU.subtract
        )
        drip_one()
        MRI = ct("MRI", [F])      # mu_r * mu_i
        nc.vector.tensor_mul(MRI[:], MU2[:, 0, :], MU2[:, 1, :])
        VRI = ct("VRI", [F])
        nc.vector.scalar_tensor_tensor(
            VRI[:], SRI[:], inv16, MRI[:], ALU.mult, ALU.subtract
        )
        drip_one()
        TAU = ct("TAU", [F])
        nc.vector.tensor_add(TAU[:], VV[:, 0, :], VV[:, 1, :])
        QD = ct("QD", [F])
        nc.vector.tensor_mul(QD[:], VV[:, 0, :], VV[:, 1, :])
        VRI2 = ct("VRI2", [F])
        nc.scalar.square(VRI2[:], VRI[:])
        drip_one()
        DET0 = ct("DET0", [F])
        nc.vector.tensor_sub(DET0[:], QD[:], VRI2[:])
        DET = ct("DET", [F])      # det = QD - VRI2 + eps*tau  (+eps^2 ~ 0)
        nc.vector.scalar_tensor_tensor(
            DET[:], TAU[:], EPS, DET0[:], ALU.mult, ALU.add
        )
        SS = ct("SS", [F])
        nc.scalar.sqrt(SS[:], DET[:])
        drip_one()
        T2 = ct("T2", [F])        # 2*s + tau
        nc.vector.scalar_tensor_tensor(
            T2[:], SS[:], 2.0, TAU[:], ALU.mult, ALU.add
        )
        TT = ct("TT", [F])        # t = sqrt(tau + 2s + 2eps)
        nc.scalar.activation(TT[:], T2[:], ACTF.Sqrt, bias=B2EPS[:])
        drip_one()
        ST = ct("ST", [F], f32)
        nc.vector.tensor_mul(ST[:], SS[:], TT[:])
        INVF = ct("INVF", [F], f32)
        nc.vector.reciprocal_approx_fast(INVF[:], ST[:])
        INV = ct("INV", [F])
        nc.scalar.copy(INV[:], INVF[:])
        drip_one()
        E12 = ct("E12", [2, F])   # (Vii + s) | (Vrr + s)
        nc.vector.tensor_add(E12[:, 0, :], VV[:, 1, :], SS[:])
        nc.vector.tensor_add(E12[:, 1, :], VV[:, 0, :], SS[:])
        drip_one()
        W2 = ct("W2", [2, F])     # Wrr | Wii
        nc.vector.tensor_mul(W2[:], E12[:], bc(INV[:], 2))
        WRIP = ct("WRIP", [F])    # +Vri*inv (true Wri = -this)
        nc.vector.tensor_mul(WRIP[:], VRI[:], INV[:])
        drip_one()
        # C4 = [[a1, a2], [b1, b2]] = G1 * (W2 repeated) - G2 * (WRIP rep4)
        MO = ct("MO", [2, 2, F])
        nc.vector.tensor_mul(MO[:], G1[:], bc(W2[:], 2, axis=1))
        ME = ct("ME", [2, 2, F])
        nc.vector.tensor_mul(ME[:], g2view, bc(bc(WRIP[:], 2), 2))
        C4 = c2pool.tile([P, 2, 2, F], f16, tag="C4", name=f"C4_{t}")
        nc.vector.tensor_sub(C4[:], MO[:], ME[:])
        drip_one()
        # AB0 = a0 | b0 = beta - (a1|b1)*mu_r - (a2|b2)*mu_i
        P1 = ct("P1", [2, F])
        nc.vector.tensor_mul(P1[:], C4[:, :, 0, :], bc(MU2[:, 0, :], 2))
        P2 = ct("P2", [2, F])
        nc.vector.tensor_mul(P2[:], C4[:, :, 1, :], bc(MU2[:, 1, :], 2))
        D0 = ct("D0", [2, F])
        nc.vector.tensor_sub(D0[:], bc(BET[:], 2), P1[:])
        AB0 = c2pool.tile([P, 2, F], f16, tag="AB0", name=f"AB0_{t}", bufs=3)
        nc.vector.tensor_sub(AB0[:], D0[:], P2[:])
        while drip:
            emit_chunk(drip.pop(0))

        # ---- pass 2: first half inline, second half deferred ----
        for h in range(NH):
            item = (t, h, XRI, C4, AB0)
            if h < NH - 4 or t == NT - 1:
                emit_chunk(item)
            else:
                deferred_p2.append(item)
    while deferred_p2:
        emit_chunk(deferred_p2.pop(0))


def build_nc() -> bacc.Bacc:
    nc = bacc.Bacc("TRN2", target_bir_lowering=False, debug=False)
    with tile.TileContext(nc) as tc:
        with ExitStack() as ctx:
            _emit(nc, ctx, tc)
    nc.compile()
    return nc


_cache: dict = {}


def _get_nc(npos: int = NPOS_FULL) -> bacc.Bacc:
    key = npos
    if key not in _cache:
        _cache[key] = build_nc()
    return _cache[key]


def make_in_maps(x_real, x_imag, gamma_rr, gamma_ri, gamma_ii, beta):
    """Shard channels across cores; pre-cast to fp16 on host."""
    in_maps = []
    for k in range(N_CORES):
        sl = slice(k * C_LOC, (k + 1) * C_LOC)
        in_maps.append(
            {
                "xr": np.ascontiguousarray(x_real[:, sl]).reshape(NB, -1).astype(np.float16),
                "xi": np.ascontiguousarray(x_imag[:, sl]).reshape(NB, -1).astype(np.float16),
                "grr": np.ascontiguousarray(gamma_rr[sl]).reshape(-1).astype(np.float16),
                "gri": np.ascontiguousarray(gamma_ri[sl]).reshape(-1).astype(np.float16),
                "gii": np.ascontiguousarray(gamma_ii[sl]).reshape(-1).astype(np.float16),
                "bet": np.ascontiguousarray(beta[sl]).reshape(-1).astype(np.float16),
            }
        )
    return in_maps


def assemble_output(results) -> np.ndarray:
    """Combine per-core planar fp16 real/imag outputs into full complex64."""
    out = np.empty((NB, C_FULL, HW), dtype=np.complex64)
    for k in range(N_CORES):
        o_r = np.asarray(results[k]["outr"]).astype(np.float32)
        o_i = np.asarray(results[k]["outi"]).astype(np.float32)
        out[:, k * C_LOC : (k + 1) * C_LOC] = (o_r + 1j * o_i).reshape(
            NB, C_LOC, HW
        )
    return out.reshape(NB, C_FULL, 256, 256)


def kernel(x_real, x_imag, gamma_rr, gamma_ri, gamma_ii, beta) -> np.ndarray:
    x_real = np.asarray(x_real, dtype=np.float32)
    x_imag = np.asarray(x_imag, dtype=np.float32)
    gamma_rr = np.asarray(gamma_rr, dtype=np.float32)
    gamma_ri = np.asarray(gamma_ri, dtype=np.float32)
    gamma_ii = np.asarray(gamma_ii, dtype=np.float32)
    beta = np.asarray(beta, dtype=np.float32)

    nc = _get_nc(NPOS_FULL)
    in_maps = make_in_maps(x_real, x_imag, gamma_rr, gamma_ri, gamma_ii, beta)
    res = run_bass_kernel_spmd(nc, in_maps, core_ids=list(range(N_CORES)))
    return assemble_output(res.results)
